# revision 36
# baseline (speedup 1.0000x reference)
"""AAM-softmax (ArcFace) loss on 8 TRN2 NeuronCores, vocab-parallel.

Math: with xn = x/|x|, wn = w/|w|, cos[b,c] = xn_b . wn_c,
  logits = 50 * (cos, except target entry replaced by phi(cos_t))
  loss = mean_b( logsumexp_c logits[b,:] - logits[b,label_b] )
Since logits <= 50, exp(50*cos) never overflows f32, so no max pass:
  S_b = sum_c exp(50*cos) - exp(50*cos_t) + exp(50*phi_t)
  loss = mean_b( ln S_b - 50*phi_t )

Layout: each core owns 12500 classes (zero-padded to 12544 = 98 tiles of
128). The host also passes the shard pre-transposed (wT, pure layout
prep) so the class tiles feed the PE stationary port directly — the
matmul runs in [class-partition, batch-free] orientation:
    psum[c, b] = sum_d wT[d, c] * xnT[d, b]       (bf16, raw W)
The per-class 1/|w_c| scale (from the natural-layout shard) and the *50
are fused into the ScalarE exp as its free per-partition scale:
    e[c, b] = exp(50 * s_c * psum[c, b])
and the class-dim reduction is a ones-stationary matmul accumulating
all 98 tiles into one PSUM row S_partial[1, b]. Per-row sums are
AllGathered; the target-row margin correction uses host-gathered
W[label] rows (pure indexing) recomputed redundantly on every core.
"""
import math

import numpy as np

from concourse import bacc, bass_isa, bass_utils, masks, mybir, tile

MARGIN = 0.1
SCALE = 50.0
COS_M = math.cos(MARGIN)
SIN_M = math.sin(MARGIN)
TH = math.cos(math.pi - MARGIN)
MM = math.sin(math.pi - MARGIN) * MARGIN

B, D, C = 1024, 256, 100000
N_CORES = 8
C_SHARD = C // N_CORES          # 12500 valid classes per core
N_CT = 98                       # class tiles of 128 (12544 padded)
C_PAD = N_CT * 128              # 12544
LAST_K = C_SHARD - 97 * 128     # 84 valid classes in the last tile
N_BB = B // 128                 # 8 batch blocks
# W is processed in batches of class tiles; the first batch is small so the
# first exp (which needs batch 0's norms) fires early.
BATCHES = [(0, 4), (4, 14), (18, 14), (32, 14), (46, 14), (60, 14), (74, 14), (88, 10)]
CT2B = {}
for _bi, (_t0, _n) in enumerate(BATCHES):
    for _t in range(_t0, _t0 + _n):
        CT2B[_t] = (_bi, _t - _t0)

F32 = mybir.dt.float32
BF16 = mybir.dt.bfloat16
AF = mybir.ActivationFunctionType
OP = mybir.AluOpType


I32 = mybir.dt.int32
RSQRT_MAGIC = 0x5F3759DF


def _rsqrt_dve(nc, pool, v, shape, name, final_scale=1.0, iters=2):
    """y ~= final_scale * rsqrt(v) entirely on VectorE (Quake bitcast seed +
    Newton iterations: 1 iter ~1.7e-3 rel err, 2 iters ~2e-6). Avoids
    ScalarE Ln/Exp table loads."""
    y = pool.tile(shape, F32, tag=f"{name}_y", name=f"{name}_y")
    t = pool.tile(shape, F32, tag=f"{name}_t", name=f"{name}_t")
    yi = y.bitcast(I32)
    nc.vector.tensor_scalar(yi, v.bitcast(I32), 1, None, op0=OP.logical_shift_right)
    nc.vector.tensor_scalar(yi, yi, RSQRT_MAGIC, -1, op0=OP.subtract, op1=OP.mult)
    for it in range(iters):
        a, b = (-0.5, 1.5) if it < iters - 1 else (-0.5 * final_scale, 1.5 * final_scale)
        nc.vector.tensor_tensor(t[:], v, y[:], op=OP.mult)
        nc.vector.tensor_tensor(t[:], t[:], y[:], op=OP.mult)
        nc.vector.tensor_scalar(t[:], t[:], a, b, op0=OP.mult, op1=OP.add)
        nc.vector.tensor_tensor(y[:], y[:], t[:], op=OP.mult)
    return y


_cached = None


def _build():
    nc = bacc.Bacc("TRN2", target_bir_lowering=False, debug=False, num_devices=N_CORES)

    x_ext = nc.dram_tensor("x", [B, D], F32, kind="ExternalInput").ap()
    w_ext = nc.dram_tensor("w", [C_PAD, D], F32, kind="ExternalInput").ap()
    wt_ext = nc.dram_tensor("wt", [D, C_PAD], BF16, kind="ExternalInput").ap()
    g_ext = nc.dram_tensor("g", [B, D], F32, kind="ExternalInput").ap()
    out_ext = nc.dram_tensor("out", [1, 1], F32, kind="ExternalOutput").ap()

    with tile.TileContext(nc) as tc:
        with (
            tc.tile_pool(name="const", bufs=1) as constp,
            tc.tile_pool(name="xp", bufs=N_BB) as xp,
            tc.tile_pool(name="gp", bufs=2) as gp,
            tc.tile_pool(name="sq", bufs=6) as sqp,
            tc.tile_pool(name="small", bufs=1) as sp,
            tc.tile_pool(name="wn", bufs=2) as wnp,
            tc.tile_pool(name="wtbf", bufs=len(BATCHES)) as wtbfp,
            tc.tile_pool(name="sosb", bufs=3 * len(BATCHES)) as sosbp,
            tc.tile_pool(name="expt", bufs=12) as exptp,
            tc.tile_pool(name="esum", bufs=6) as esump,
            tc.tile_pool(name="dram", bufs=1, space="DRAM") as dramp,
        ):
            ident = constp.tile([128, 128], BF16, tag="ident")
            masks.make_identity(nc, ident[:])
            ones_bf = constp.tile([128, 1], BF16, tag="ones_bf")
            nc.vector.memset(ones_bf[:], 1.0)
            # activation() lowers float biases through nc.const_aps
            czero = constp.tile([128, 1], F32, tag="czero")
            nc.vector.memset(czero[:], 0.0)
            ceps = constp.tile([128, 1], F32, tag="ceps")
            nc.vector.memset(ceps[:], 1e-30)
            nc.const_aps.aps[(F32, 0.0)] = czero[:]
            nc.const_aps.aps[(F32, 1e-30)] = ceps[:]

            # ---------- x prep: xn (bf16) transposed to [d, b] layout ----------
            x_tiles = []
            sosx = sp.tile([128, N_BB], F32, tag="sosx")
            for bb in range(N_BB):
                xt = xp.tile([128, D], F32, tag="xt", name=f"xt{bb}")
                nc.sync.dma_start(xt[:], x_ext[bb * 128:(bb + 1) * 128, :])
                x_tiles.append(xt)
                sq = sqp.tile([128, D], F32, tag="sq", name=f"sqx{bb}")
                nc.vector.scalar_tensor_tensor(
                    out=sq[:], in0=xt[:], scalar=1.0, in1=xt[:],
                    op0=OP.mult, op1=OP.mult, accum_out=sosx[:, bb:bb + 1])
            sx = _rsqrt_dve(nc, sp, sosx[:], [128, N_BB], "sx")

            xnt = sp.tile([128, 2, B], BF16, tag="xnt")
            with tc.tile_pool(name="pst", bufs=2, space="PSUM") as pstp:
                for bb in range(N_BB):
                    xnb = sqp.tile([128, D], BF16, tag="xnb", name=f"xnb{bb}")
                    nc.vector.tensor_scalar_mul(xnb[:], x_tiles[bb][:], sx[:, bb:bb + 1])
                    for dc in range(2):
                        pt = pstp.tile([128, 128], BF16, tag="pst", name=f"pst{bb}_{dc}")
                        nc.tensor.transpose(pt[:], xnb[:, dc * 128:(dc + 1) * 128], ident[:])
                        nc.vector.tensor_copy(xnt[:, dc, bb * 128:(bb + 1) * 128], pt[:])

            # ---------- W natural + wT, interleaved so both chains start early ----------
            s_tiles = []
            wtbf_tiles = []
            for sb, (t0, nt) in enumerate(BATCHES):
                wn = wnp.tile([128, 14, D], F32, tag="wn", name=f"wn{sb}")
                src = w_ext[t0 * 128:(t0 + nt) * 128, :].rearrange("(t p) d -> p t d", p=128)
                nc.sync.dma_start(wn[:, :nt], src)
                sosb = sosbp.tile([128, 14], F32, tag="sosb", name=f"sosb{sb}")
                for t in range(nt):
                    sq = sqp.tile([128, D], F32, tag="sq", name=f"sqw{sb}_{t}")
                    nc.vector.scalar_tensor_tensor(
                        out=sq[:], in0=wn[:, t], scalar=1.0, in1=wn[:, t],
                        op0=OP.mult, op1=OP.mult, accum_out=sosb[:, t:t + 1])
                # sv = 50/|w_c| — the full per-class exp scale, DVE-only
                sv = _rsqrt_dve(nc, sosbp, sosb[:, :nt], [128, nt], f"sv{sb}", final_scale=SCALE, iters=1)
                s_tiles.append(sv)

                wtbf = wtbfp.tile([128, 2, 14 * 128], BF16, tag="wtbf", name=f"wtbf{sb}")
                c0, cw = t0 * 128, nt * 128
                for dc in range(2):
                    nc.sync.dma_start(wtbf[:, dc, :cw], wt_ext[dc * 128:(dc + 1) * 128, c0:c0 + cw])
                wtbf_tiles.append(wtbf)

            # ---------- g prep: target-row margin terms ----------
            sosg = sp.tile([128, N_BB], F32, tag="sosg")
            rd = sp.tile([128, N_BB], F32, tag="rd")
            for bb in range(N_BB):
                gt = gp.tile([128, D], F32, tag="gt", name=f"gt{bb}")
                nc.sync.dma_start(gt[:], g_ext[bb * 128:(bb + 1) * 128, :])
                sq = sqp.tile([128, D], F32, tag="sq", name=f"sqg{bb}")
                nc.vector.scalar_tensor_tensor(
                    out=sq[:], in0=gt[:], scalar=1.0, in1=gt[:],
                    op0=OP.mult, op1=OP.mult, accum_out=sosg[:, bb:bb + 1])
                sq2 = sqp.tile([128, D], F32, tag="sq", name=f"sqxg{bb}")
                nc.vector.scalar_tensor_tensor(
                    out=sq2[:], in0=x_tiles[bb][:], scalar=1.0, in1=gt[:],
                    op0=OP.mult, op1=OP.mult, accum_out=rd[:, bb:bb + 1])
            sg = _rsqrt_dve(nc, sp, sosg[:], [128, N_BB], "sg")

            # cos_t = rd * sx * sg, then phi with hard-margin fallback
            cos_t = sp.tile([128, N_BB], F32, tag="cos_t")
            nc.vector.tensor_tensor(cos_t[:], rd[:], sx[:], op=OP.mult)
            nc.vector.tensor_tensor(cos_t[:], cos_t[:], sg[:], op=OP.mult)

            om = sp.tile([128, N_BB], F32, tag="om")
            nc.vector.scalar_tensor_tensor(om[:], cos_t[:], 1.0, cos_t[:], op0=OP.mult, op1=OP.mult)
            nc.vector.tensor_scalar(om[:], om[:], -1.0, 1.0, op0=OP.mult, op1=OP.add)
            nc.vector.tensor_scalar_max(om[:], om[:], 0.0)
            rsq_om = _rsqrt_dve(nc, sp, om[:], [128, N_BB], "rsq_om")
            sin_t = sp.tile([128, N_BB], F32, tag="sin_t")
            nc.vector.tensor_tensor(sin_t[:], om[:], rsq_om[:], op=OP.mult)

            phi = sp.tile([128, N_BB], F32, tag="phi")
            nc.vector.tensor_scalar_mul(phi[:], sin_t[:], SIN_M)
            nc.vector.scalar_tensor_tensor(phi[:], cos_t[:], COS_M, phi[:], op0=OP.mult, op1=OP.subtract)
            mask = sp.tile([128, N_BB], mybir.dt.uint8, tag="mask")
            nc.vector.tensor_scalar(mask[:], cos_t[:], TH, None, op0=OP.is_gt)
            alt = sp.tile([128, N_BB], F32, tag="alt")
            nc.vector.tensor_scalar_sub(alt[:], cos_t[:], MM)
            phi_f = sp.tile([128, N_BB], F32, tag="phi_f")
            nc.vector.select(phi_f[:], mask[:], phi[:], alt[:])

            delta = sp.tile([128, N_BB], F32, tag="delta")
            expc = sp.tile([128, N_BB], F32, tag="expc")
            nc.scalar.activation(delta[:], phi_f[:], AF.Exp, scale=SCALE)
            nc.scalar.activation(expc[:], cos_t[:], AF.Exp, scale=SCALE)
            nc.vector.tensor_sub(delta[:], delta[:], expc[:])

            ag_inA = dramp.tile([1, B], F32, tag="ag_inA")
            ag_outA = dramp.tile([N_CORES, B], F32, tag="ag_outA")
            s_all = sp.tile([128, N_BB, 2 * N_CORES], F32, tag="s_all")

            # ---------- main: matmul -> exp(scale=50/|w|) -> class-sum matmul ----------
            with tc.tile_pool(name="psm", bufs=3, space="PSUM") as psmp, \
                 tc.tile_pool(name="spsm", bufs=1, space="PSUM") as spsmp:
                # four independent class-sum accumulator rows at partitions
                # 0/32/64/96: sum-matmuls for 4 consecutive tiles target
                # disjoint PE column groups and execute concurrently
                spsum = spsmp.tile([128, B], F32, tag="spsum")
                spsumB = spsum
                pend = []
                HALF = 48

                def flush_sums():
                    # bh-major so consecutive matmuls hit different PE column
                    # groups and overlap in the array
                    for bh in range(2):
                        for j, (eo_t, kk, ct_t) in enumerate(pend):
                            nc.tensor.matmul(
                                spsum[32 * j:32 * j + 1, bh * 512:(bh + 1) * 512],
                                ones_bf[:kk, :],
                                eo_t[:kk, bh * 512:(bh + 1) * 512],
                                start=(ct_t < 4 or HALF <= ct_t < HALF + 4),
                                stop=(HALF - 4 <= ct_t < HALF or ct_t + 4 >= N_CT),
                                skip_group_check=True, tile_position=(0, 32 * j))
                    pend.clear()

                def drain_half(acc, dst):
                    # parallel 2-level tree: DVE and ScalarE each fold one PSUM
                    # row pair (tensor ops may read PSUM as one operand), then
                    # one DVE add combines. 3 serial steps instead of 7.
                    nm = dst.tensor.name
                    t01 = sp.tile([1, B], F32, tag=f"t01_{nm}", name=f"t01_{nm}")
                    t23 = sp.tile([1, B], F32, tag=f"t23_{nm}", name=f"t23_{nm}")
                    nc.vector.tensor_copy(t01[:], acc[0:1, :])
                    nc.scalar.copy(t23[:], acc[64:65, :])
                    nc.vector.tensor_tensor(t01[:], t01[:], acc[32:33, :], op=OP.add)
                    nc.vector.tensor_tensor(t23[:], t23[:], acc[96:97, :], op=OP.add)
                    nc.vector.tensor_tensor(dst[:], t01[:], t23[:], op=OP.add)

                scopyA = sp.tile([1, B], F32, tag="scopyA")
                scopyB = sp.tile([1, B], F32, tag="scopyB")
                for ct in range(N_CT):
                    wg, loc = CT2B[ct]
                    cl = loc * 128
                    lhs = wtbf_tiles[wg]
                    ps = psmp.tile([128, B], F32, tag="psm", name=f"ps{ct}")
                    for dc in range(2):
                        for bh in range(2):
                            nc.tensor.matmul(
                                ps[:, bh * 512:(bh + 1) * 512],
                                lhs[:, dc, cl:cl + 128],
                                xnt[:, dc, bh * 512:(bh + 1) * 512],
                                start=(dc == 0), stop=(dc == 1))
                    eo = exptp.tile([128, B], BF16, tag="expt", name=f"eo{ct}")
                    nc.scalar.activation(eo[:], ps[:], AF.Exp,
                                         scale=s_tiles[wg][:, loc:loc + 1])
                    pend.append((eo, LAST_K if ct == N_CT - 1 else 128, ct))
                    if len(pend) == 4:
                        flush_sums()
                    if ct == HALF - 1:
                        # first-half sums complete: AllGather them mid-compute so
                        # the collective's latency/skew hides under the rest
                        drain_half(spsum, scopyA)
                        nc.sync.dma_start(ag_inA[:], scopyA[:])
                        nc.gpsimd.collective_compute(
                            "AllGather", OP.bypass,
                            replica_groups=[list(range(N_CORES))],
                            ins=[ag_inA.opt()], outs=[ag_outA.opt()])
                        for r in range(N_CORES):
                            nc.sync.dma_start(
                                s_all[:, :, r],
                                ag_outA[r:r + 1, :].rearrange("one (o p) -> p (one o)", p=128))
                flush_sums()
                drain_half(spsumB, scopyB)

            ag_inB = dramp.tile([1, B], F32, tag="ag_inB")
            ag_outB = dramp.tile([N_CORES, B], F32, tag="ag_outB")
            nc.sync.dma_start(ag_inB[:], scopyB[:])
            nc.gpsimd.collective_compute(
                "AllGather", OP.bypass,
                replica_groups=[list(range(N_CORES))],
                ins=[ag_inB.opt()], outs=[ag_outB.opt()])
            # warm the Ln table while the collective is in flight
            lnwarm = sp.tile([1, 1], F32, tag="lnwarm")
            nc.scalar.activation(lnwarm[:], ceps[0:1, :], AF.Ln, bias=1e-30)
            for r in range(N_CORES):
                nc.sync.dma_start(s_all[:, :, N_CORES + r],
                                  ag_outB[r:r + 1, :].rearrange("one (o p) -> p (one o)", p=128))

            s_sum = sp.tile([128, N_BB], F32, tag="s_sum")
            nc.vector.reduce_sum(s_sum[:], s_all[:], axis=mybir.AxisListType.X)
            nc.vector.tensor_tensor(s_sum[:], s_sum[:], delta[:], op=OP.add)
            lnS = sp.tile([128, N_BB], F32, tag="lnS")
            nc.scalar.activation(lnS[:], s_sum[:], AF.Ln)
            negl = sp.tile([128, N_BB], F32, tag="negl")
            nc.vector.scalar_tensor_tensor(negl[:], phi_f[:], SCALE, lnS[:], op0=OP.mult, op1=OP.subtract)
            col = sp.tile([128, 1], F32, tag="col")
            nc.vector.reduce_sum(col[:], negl[:], axis=mybir.AxisListType.X)
            tot = sp.tile([128, 1], F32, tag="tot")
            nc.gpsimd.partition_all_reduce(tot[:], col[:], channels=128, reduce_op=bass_isa.ReduceOp.add)
            loss = sp.tile([1, 1], F32, tag="loss")
            nc.vector.tensor_scalar_mul(loss[:], tot[0:1, :], -1.0 / B)
            nc.sync.dma_start(out_ext[:, :], loss[:])

    nc.compile()
    return nc


def _to_bf16_t(ws):
    """Transpose + round-to-nearest-even bf16 cast, returned as uint16-backed
    bf16 array (run_bass_kernel_spmd matches by buffer bytes)."""
    a = np.ascontiguousarray(ws.T)
    u = a.view(np.uint32)
    rounded = ((u + 0x7FFF + ((u >> 16) & 1)) >> 16).astype(np.uint16)
    try:
        import ml_dtypes
        return rounded.view(ml_dtypes.bfloat16)
    except ImportError:
        return rounded


def _get_nc():
    global _cached
    if _cached is None:
        _cached = _build()
    return _cached


def run(x, weight, label, trace=False, trace_cores=None):
    nc = _get_nc()
    x = np.ascontiguousarray(np.asarray(x, dtype=np.float32))
    w = np.ascontiguousarray(np.asarray(weight, dtype=np.float32))
    lab = np.asarray(label).astype(np.int64)
    g = np.ascontiguousarray(w[lab])
    in_maps = []
    for i in range(N_CORES):
        ws = np.zeros((C_PAD, D), np.float32)
        ws[:C_SHARD] = w[i * C_SHARD:(i + 1) * C_SHARD]
        in_maps.append({"x": x, "w": ws, "wt": _to_bf16_t(ws), "g": g})
    res = bass_utils.run_bass_kernel_spmd(
        nc, in_maps, core_ids=list(range(N_CORES)), trace=trace,
        **({"trace_cores": trace_cores} if trace_cores else {}))
    loss = np.array(res.results[0]["out"][0, 0], dtype=np.float32)
    return loss, res


def kernel(x, weight, label):
    loss, _ = run(x, weight, label, trace=False)
    return loss


# revision 38
# speedup vs baseline: 1.3368x; 1.3368x over previous
"""AAM-softmax (ArcFace) loss on 8 TRN2 NeuronCores, vocab-parallel.

Math: with xn = x/|x|, wn = w/|w|, cos[b,c] = xn_b . wn_c,
  logits = 50 * (cos, except target entry replaced by phi(cos_t))
  loss = mean_b( logsumexp_c logits[b,:] - logits[b,label_b] )
Since logits <= 50, exp(50*cos) never overflows f32, so no max pass:
  S_b = sum_c exp(50*cos) - exp(50*cos_t) + exp(50*phi_t)
  loss = mean_b( ln S_b - 50*phi_t )

Layout: each core owns 12500 classes (zero-padded to 12544 = 98 tiles of
128). The host also passes the shard pre-transposed (wT, pure layout
prep) so the class tiles feed the PE stationary port directly — the
matmul runs in [class-partition, batch-free] orientation:
    psum[c, b] = sum_d wT[d, c] * xnT[d, b]       (bf16, raw W)
The per-class 1/|w_c| scale (from the natural-layout shard) and the *50
are fused into the ScalarE exp as its free per-partition scale:
    e[c, b] = exp(50 * s_c * psum[c, b])
and the class-dim reduction is a ones-stationary matmul accumulating
all 98 tiles into one PSUM row S_partial[1, b]. Per-row sums are
AllGathered; the target-row margin correction uses host-gathered
W[label] rows (pure indexing) recomputed redundantly on every core.
"""
import math

import numpy as np

from concourse import bacc, bass_isa, bass_utils, masks, mybir, tile

MARGIN = 0.1
SCALE = 50.0
COS_M = math.cos(MARGIN)
SIN_M = math.sin(MARGIN)
TH = math.cos(math.pi - MARGIN)
MM = math.sin(math.pi - MARGIN) * MARGIN

B, D, C = 1024, 256, 100000
N_CORES = 8
C_SHARD = C // N_CORES          # 12500 valid classes per core
N_CT = 98                       # class tiles of 128 (12544 padded)
C_PAD = N_CT * 128              # 12544
LAST_K = C_SHARD - 97 * 128     # 84 valid classes in the last tile
N_BB = B // 128                 # 8 batch blocks
# W is processed in batches of class tiles; the first batch is small so the
# first exp (which needs batch 0's norms) fires early.
BATCHES = [(0, 4), (4, 14), (18, 14), (32, 14), (46, 14), (60, 14), (74, 14), (88, 10)]
CT2B = {}
for _bi, (_t0, _n) in enumerate(BATCHES):
    for _t in range(_t0, _t0 + _n):
        CT2B[_t] = (_bi, _t - _t0)

F32 = mybir.dt.float32
BF16 = mybir.dt.bfloat16
AF = mybir.ActivationFunctionType
OP = mybir.AluOpType


I32 = mybir.dt.int32
RSQRT_MAGIC = 0x5F3759DF


def _rsqrt_dve(nc, pool, v, shape, name, final_scale=1.0, iters=2):
    """y ~= final_scale * rsqrt(v) entirely on VectorE (Quake bitcast seed +
    Newton iterations: 1 iter ~1.7e-3 rel err, 2 iters ~2e-6). Avoids
    ScalarE Ln/Exp table loads."""
    y = pool.tile(shape, F32, tag=f"{name}_y", name=f"{name}_y")
    t = pool.tile(shape, F32, tag=f"{name}_t", name=f"{name}_t")
    yi = y.bitcast(I32)
    nc.vector.tensor_scalar(yi, v.bitcast(I32), 1, None, op0=OP.logical_shift_right)
    nc.vector.tensor_scalar(yi, yi, RSQRT_MAGIC, -1, op0=OP.subtract, op1=OP.mult)
    for it in range(iters):
        a, b = (-0.5, 1.5) if it < iters - 1 else (-0.5 * final_scale, 1.5 * final_scale)
        nc.vector.tensor_tensor(t[:], v, y[:], op=OP.mult)
        nc.vector.tensor_tensor(t[:], t[:], y[:], op=OP.mult)
        nc.vector.tensor_scalar(t[:], t[:], a, b, op0=OP.mult, op1=OP.add)
        nc.vector.tensor_tensor(y[:], y[:], t[:], op=OP.mult)
    return y


_cached = None


def _build():
    nc = bacc.Bacc("TRN2", target_bir_lowering=False, debug=False, num_devices=N_CORES)

    x_ext = nc.dram_tensor("x", [B, D], F32, kind="ExternalInput").ap()
    w_ext = nc.dram_tensor("w", [C_PAD, D], F32, kind="ExternalInput").ap()
    wt_ext = nc.dram_tensor("wt", [D, C_PAD], BF16, kind="ExternalInput").ap()
    g_ext = nc.dram_tensor("g", [B, D], F32, kind="ExternalInput").ap()
    out_ext = nc.dram_tensor("out", [1, 1], F32, kind="ExternalOutput").ap()

    with tile.TileContext(nc) as tc:
        with (
            tc.tile_pool(name="const", bufs=1) as constp,
            tc.tile_pool(name="xp", bufs=N_BB) as xp,
            tc.tile_pool(name="gp", bufs=2) as gp,
            tc.tile_pool(name="sq", bufs=6) as sqp,
            tc.tile_pool(name="small", bufs=1) as sp,
            tc.tile_pool(name="wn", bufs=2) as wnp,
            tc.tile_pool(name="wtbf", bufs=len(BATCHES)) as wtbfp,
            tc.tile_pool(name="sosb", bufs=3 * len(BATCHES)) as sosbp,
            tc.tile_pool(name="expt", bufs=12) as exptp,
            tc.tile_pool(name="esum", bufs=6) as esump,
            tc.tile_pool(name="dram", bufs=1, space="DRAM") as dramp,
        ):
            ident = constp.tile([128, 128], BF16, tag="ident")
            masks.make_identity(nc, ident[:])
            ones_bf = constp.tile([128, 1], BF16, tag="ones_bf")
            nc.vector.memset(ones_bf[:], 1.0)
            # activation() lowers float biases through nc.const_aps
            czero = constp.tile([128, 1], F32, tag="czero")
            nc.vector.memset(czero[:], 0.0)
            ceps = constp.tile([128, 1], F32, tag="ceps")
            nc.vector.memset(ceps[:], 1e-30)
            nc.const_aps.aps[(F32, 0.0)] = czero[:]
            nc.const_aps.aps[(F32, 1e-30)] = ceps[:]

            # ---------- x prep: xn (bf16) transposed to [d, b] layout ----------
            x_tiles = []
            sosx = sp.tile([128, N_BB], F32, tag="sosx")
            for bb in range(N_BB):
                xt = xp.tile([128, D], F32, tag="xt", name=f"xt{bb}")
                nc.sync.dma_start(xt[:], x_ext[bb * 128:(bb + 1) * 128, :])
                x_tiles.append(xt)
                sq = sqp.tile([128, D], F32, tag="sq", name=f"sqx{bb}")
                nc.vector.scalar_tensor_tensor(
                    out=sq[:], in0=xt[:], scalar=1.0, in1=xt[:],
                    op0=OP.mult, op1=OP.mult, accum_out=sosx[:, bb:bb + 1])
            sx = _rsqrt_dve(nc, sp, sosx[:], [128, N_BB], "sx")

            xnt = sp.tile([128, 2, B], BF16, tag="xnt")
            with tc.tile_pool(name="pst", bufs=2, space="PSUM") as pstp:
                for bb in range(N_BB):
                    xnb = sqp.tile([128, D], BF16, tag="xnb", name=f"xnb{bb}")
                    nc.vector.tensor_scalar_mul(xnb[:], x_tiles[bb][:], sx[:, bb:bb + 1])
                    for dc in range(2):
                        pt = pstp.tile([128, 128], BF16, tag="pst", name=f"pst{bb}_{dc}")
                        nc.tensor.transpose(pt[:], xnb[:, dc * 128:(dc + 1) * 128], ident[:])
                        nc.vector.tensor_copy(xnt[:, dc, bb * 128:(bb + 1) * 128], pt[:])

            # ---------- W natural + wT, interleaved so both chains start early ----------
            s_tiles = []
            wtbf_tiles = []
            for sb, (t0, nt) in enumerate(BATCHES):
                wn = wnp.tile([128, 14, D], F32, tag="wn", name=f"wn{sb}")
                src = w_ext[t0 * 128:(t0 + nt) * 128, :].rearrange("(t p) d -> p t d", p=128)
                nc.sync.dma_start(wn[:, :nt], src)
                sosb = sosbp.tile([128, 14], F32, tag="sosb", name=f"sosb{sb}")
                for t in range(nt):
                    sq = sqp.tile([128, D], F32, tag="sq", name=f"sqw{sb}_{t}")
                    nc.vector.scalar_tensor_tensor(
                        out=sq[:], in0=wn[:, t], scalar=1.0, in1=wn[:, t],
                        op0=OP.mult, op1=OP.mult, accum_out=sosb[:, t:t + 1])
                # sv = 50/|w_c| — the full per-class exp scale, DVE-only
                sv = _rsqrt_dve(nc, sosbp, sosb[:, :nt], [128, nt], f"sv{sb}", final_scale=SCALE, iters=1)
                s_tiles.append(sv)

                wtbf = wtbfp.tile([128, 2, 14 * 128], BF16, tag="wtbf", name=f"wtbf{sb}")
                c0, cw = t0 * 128, nt * 128
                for dc in range(2):
                    nc.sync.dma_start(wtbf[:, dc, :cw], wt_ext[dc * 128:(dc + 1) * 128, c0:c0 + cw])
                wtbf_tiles.append(wtbf)

            # ---------- g prep: target-row margin terms ----------
            sosg = sp.tile([128, N_BB], F32, tag="sosg")
            rd = sp.tile([128, N_BB], F32, tag="rd")
            for bb in range(N_BB):
                gt = gp.tile([128, D], F32, tag="gt", name=f"gt{bb}")
                nc.sync.dma_start(gt[:], g_ext[bb * 128:(bb + 1) * 128, :])
                sq = sqp.tile([128, D], F32, tag="sq", name=f"sqg{bb}")
                nc.vector.scalar_tensor_tensor(
                    out=sq[:], in0=gt[:], scalar=1.0, in1=gt[:],
                    op0=OP.mult, op1=OP.mult, accum_out=sosg[:, bb:bb + 1])
                sq2 = sqp.tile([128, D], F32, tag="sq", name=f"sqxg{bb}")
                nc.vector.scalar_tensor_tensor(
                    out=sq2[:], in0=x_tiles[bb][:], scalar=1.0, in1=gt[:],
                    op0=OP.mult, op1=OP.mult, accum_out=rd[:, bb:bb + 1])
            sg = _rsqrt_dve(nc, sp, sosg[:], [128, N_BB], "sg")

            # cos_t = rd * sx * sg, then phi with hard-margin fallback
            cos_t = sp.tile([128, N_BB], F32, tag="cos_t")
            nc.vector.tensor_tensor(cos_t[:], rd[:], sx[:], op=OP.mult)
            nc.vector.tensor_tensor(cos_t[:], cos_t[:], sg[:], op=OP.mult)

            om = sp.tile([128, N_BB], F32, tag="om")
            nc.vector.scalar_tensor_tensor(om[:], cos_t[:], 1.0, cos_t[:], op0=OP.mult, op1=OP.mult)
            nc.vector.tensor_scalar(om[:], om[:], -1.0, 1.0, op0=OP.mult, op1=OP.add)
            nc.vector.tensor_scalar_max(om[:], om[:], 0.0)
            rsq_om = _rsqrt_dve(nc, sp, om[:], [128, N_BB], "rsq_om")
            sin_t = sp.tile([128, N_BB], F32, tag="sin_t")
            nc.vector.tensor_tensor(sin_t[:], om[:], rsq_om[:], op=OP.mult)

            phi = sp.tile([128, N_BB], F32, tag="phi")
            nc.vector.tensor_scalar_mul(phi[:], sin_t[:], SIN_M)
            nc.vector.scalar_tensor_tensor(phi[:], cos_t[:], COS_M, phi[:], op0=OP.mult, op1=OP.subtract)
            mask = sp.tile([128, N_BB], mybir.dt.uint8, tag="mask")
            nc.vector.tensor_scalar(mask[:], cos_t[:], TH, None, op0=OP.is_gt)
            alt = sp.tile([128, N_BB], F32, tag="alt")
            nc.vector.tensor_scalar_sub(alt[:], cos_t[:], MM)
            phi_f = sp.tile([128, N_BB], F32, tag="phi_f")
            nc.vector.select(phi_f[:], mask[:], phi[:], alt[:])

            delta = sp.tile([128, N_BB], F32, tag="delta")
            expc = sp.tile([128, N_BB], F32, tag="expc")
            nc.scalar.activation(delta[:], phi_f[:], AF.Exp, scale=SCALE)
            nc.scalar.activation(expc[:], cos_t[:], AF.Exp, scale=SCALE)
            nc.vector.tensor_sub(delta[:], delta[:], expc[:])

            ag_inA = dramp.tile([1, B], F32, tag="ag_inA")
            ag_outA = dramp.tile([N_CORES, B], F32, tag="ag_outA")
            s_all = sp.tile([128, N_BB, 2 * N_CORES], F32, tag="s_all")

            # ---------- main: matmul -> exp(scale=50/|w|) -> class-sum matmul ----------
            with tc.tile_pool(name="psm", bufs=3, space="PSUM") as psmp, \
                 tc.tile_pool(name="spsm", bufs=1, space="PSUM") as spsmp:
                # four independent class-sum accumulator rows at partitions
                # 0/32/64/96: sum-matmuls for 4 consecutive tiles target
                # disjoint PE column groups and execute concurrently
                spsum = spsmp.tile([128, B], F32, tag="spsum")
                spsumB = spsum
                pend = []
                HALF = 48

                def flush_sums():
                    # bh-major so consecutive matmuls hit different PE column
                    # groups and overlap in the array
                    for bh in range(2):
                        for j, (eo_t, kk, ct_t) in enumerate(pend):
                            nc.tensor.matmul(
                                spsum[32 * j:32 * j + 1, bh * 512:(bh + 1) * 512],
                                ones_bf[:kk, :],
                                eo_t[:kk, bh * 512:(bh + 1) * 512],
                                start=(ct_t < 4 or HALF <= ct_t < HALF + 4),
                                stop=(HALF - 4 <= ct_t < HALF or ct_t + 4 >= N_CT),
                                skip_group_check=True, tile_position=(0, 32 * j))
                    pend.clear()

                def drain_half(acc, dst):
                    # parallel 2-level tree: DVE and ScalarE each fold one PSUM
                    # row pair (tensor ops may read PSUM as one operand), then
                    # one DVE add combines. 3 serial steps instead of 7.
                    nm = dst.tensor.name
                    t01 = sp.tile([1, B], F32, tag=f"t01_{nm}", name=f"t01_{nm}")
                    t23 = sp.tile([1, B], F32, tag=f"t23_{nm}", name=f"t23_{nm}")
                    nc.vector.tensor_copy(t01[:], acc[0:1, :])
                    nc.scalar.copy(t23[:], acc[64:65, :])
                    nc.vector.tensor_tensor(t01[:], t01[:], acc[32:33, :], op=OP.add)
                    nc.vector.tensor_tensor(t23[:], t23[:], acc[96:97, :], op=OP.add)
                    nc.vector.tensor_tensor(dst[:], t01[:], t23[:], op=OP.add)

                scopyA = sp.tile([1, B], F32, tag="scopyA")
                scopyB = sp.tile([1, B], F32, tag="scopyB")
                for ct in range(N_CT):
                    wg, loc = CT2B[ct]
                    cl = loc * 128
                    lhs = wtbf_tiles[wg]
                    ps = psmp.tile([128, B], F32, tag="psm", name=f"ps{ct}")
                    for dc in range(2):
                        for bh in range(2):
                            nc.tensor.matmul(
                                ps[:, bh * 512:(bh + 1) * 512],
                                lhs[:, dc, cl:cl + 128],
                                xnt[:, dc, bh * 512:(bh + 1) * 512],
                                start=(dc == 0), stop=(dc == 1))
                    eo = exptp.tile([128, B], BF16, tag="expt", name=f"eo{ct}")
                    nc.scalar.activation(eo[:], ps[:], AF.Exp,
                                         scale=s_tiles[wg][:, loc:loc + 1])
                    pend.append((eo, LAST_K if ct == N_CT - 1 else 128, ct))
                    if len(pend) == 4:
                        flush_sums()
                    if ct == HALF - 1:
                        # first-half sums complete: AllGather them mid-compute so
                        # the collective's latency/skew hides under the rest
                        drain_half(spsum, scopyA)
                        nc.sync.dma_start(ag_inA[:], scopyA[:])
                        nc.gpsimd.collective_compute(
                            "AllGather", OP.bypass,
                            replica_groups=[list(range(N_CORES))],
                            ins=[ag_inA.opt()], outs=[ag_outA.opt()])
                        for r in range(N_CORES):
                            nc.sync.dma_start(
                                s_all[:, :, r],
                                ag_outA[r:r + 1, :].rearrange("one (o p) -> p (one o)", p=128))
                flush_sums()
                drain_half(spsumB, scopyB)

            ag_inB = dramp.tile([1, B], F32, tag="ag_inB")
            ag_outB = dramp.tile([N_CORES, B], F32, tag="ag_outB")
            nc.sync.dma_start(ag_inB[:], scopyB[:])
            nc.gpsimd.collective_compute(
                "AllGather", OP.bypass,
                replica_groups=[list(range(N_CORES))],
                ins=[ag_inB.opt()], outs=[ag_outB.opt()])
            # warm the Ln table while the collective is in flight
            lnwarm = sp.tile([1, 1], F32, tag="lnwarm")
            nc.scalar.activation(lnwarm[:], ceps[0:1, :], AF.Ln, bias=1e-30)
            for r in range(N_CORES):
                nc.sync.dma_start(s_all[:, :, N_CORES + r],
                                  ag_outB[r:r + 1, :].rearrange("one (o p) -> p (one o)", p=128))

            s_sum = sp.tile([128, N_BB], F32, tag="s_sum")
            nc.vector.reduce_sum(s_sum[:], s_all[:], axis=mybir.AxisListType.X)
            nc.vector.tensor_tensor(s_sum[:], s_sum[:], delta[:], op=OP.add)
            lnS = sp.tile([128, N_BB], F32, tag="lnS")
            nc.scalar.activation(lnS[:], s_sum[:], AF.Ln)
            negl = sp.tile([128, N_BB], F32, tag="negl")
            nc.vector.scalar_tensor_tensor(negl[:], phi_f[:], SCALE, lnS[:], op0=OP.mult, op1=OP.subtract)
            col = sp.tile([128, 1], F32, tag="col")
            nc.vector.reduce_sum(col[:], negl[:], axis=mybir.AxisListType.X)
            tot = sp.tile([128, 1], F32, tag="tot")
            nc.gpsimd.partition_all_reduce(tot[:], col[:], channels=128, reduce_op=bass_isa.ReduceOp.add)
            loss = sp.tile([1, 1], F32, tag="loss")
            nc.vector.tensor_scalar_mul(loss[:], tot[0:1, :], -1.0 / B)
            nc.sync.dma_start(out_ext[:, :], loss[:])

    nc.compile()
    return nc


def _to_bf16_t(ws):
    """Transpose + round-to-nearest-even bf16 cast, returned as uint16-backed
    bf16 array (run_bass_kernel_spmd matches by buffer bytes)."""
    a = np.ascontiguousarray(ws.T)
    u = a.view(np.uint32)
    rounded = ((u + 0x7FFF + ((u >> 16) & 1)) >> 16).astype(np.uint16)
    try:
        import ml_dtypes
        return rounded.view(ml_dtypes.bfloat16)
    except ImportError:
        return rounded


def _get_nc():
    global _cached
    if _cached is None:
        _cached = _build()
    return _cached


def run(x, weight, label, trace=False, trace_cores=None):
    nc = _get_nc()
    x = np.ascontiguousarray(np.asarray(x, dtype=np.float32))
    w = np.ascontiguousarray(np.asarray(weight, dtype=np.float32))
    lab = np.asarray(label).astype(np.int64)
    g = np.ascontiguousarray(w[lab])
    in_maps = []
    for i in range(N_CORES):
        ws = np.zeros((C_PAD, D), np.float32)
        ws[:C_SHARD] = w[i * C_SHARD:(i + 1) * C_SHARD]
        in_maps.append({"x": x, "w": ws, "wt": _to_bf16_t(ws), "g": g})
    res = bass_utils.run_bass_kernel_spmd(
        nc, in_maps, core_ids=list(range(N_CORES)), trace=trace,
        **({"trace_cores": trace_cores} if trace_cores else {}))
    loss = np.array(res.results[0]["out"][0, 0], dtype=np.float32)
    return loss, res


def kernel(x, weight, label):
    loss, _ = run(x, weight, label, trace=False)
    return loss
